# revision 4
# baseline (speedup 1.0000x reference)
"""Trainium2 Bass kernel for the KnowledgeRetrievalNetwork (spiking MoE).

Strategy: expert-parallel across 8 NeuronCores (E=8, one expert per core).

The reference computes, with fixed PRNG key 42:
    qs   = bernoulli(k0, clip(mq,0,1))                    [B,D]
    gate = softmax(qs @ Wg.T + bg)                        [B,E]
    lin1 = qs @ W1[e].T ; s1 = bernoulli(k1, sigmoid(5*(lin1-th1)))
    lin2 = s1 @ W2[e].T ; s2 = bernoulli(k2, sigmoid(5*(lin2-th2)))
    out  = sum_e gate[:,e] * s2[e]                        [B,H]

jax's bernoulli(key,p) is `uniform(key,shape) < p` and threefry is
backend-invariant, so the uniform draws are input-independent constants.
We precompute them on host and fold them into per-element thresholds:
    u < sigmoid(SLOPE*(lin-th))  <=>  lin > th + logit(u)/SLOPE
The device kernel is then two dense GEMM+compare layers per expert.

Weights use an fp16 hi+lo split (w = hi + lo exactly at ~2^-22 relative):
both halves accumulate into the same PSUM bank at full bf16-rate on the PE,
giving near-fp32 matmul precision at 2 cycles/row instead of fp32's 4.

Everything is computed in a transposed [feature, batch] layout so layer 2
consumes layer 1's output directly (contraction dim on partitions, no
transposes anywhere).
"""

import numpy as np

B, D, H, E = 4096, 2048, 2048, 8
SLOPE = 5.0
P = 128                    # partitions
NB = 512                   # batch tile (one PSUM bank of fp32)
DC = D // P                # 16 contraction chunks (layer 1)
HB = H // P                # 16 output blocks
BT = B // NB               # 8 batch tiles

_cache = {}


def _logit_thresholds():
    """Input-independent transposed logit(u)/SLOPE terms, [E][P, HB, B] f32."""
    if "L1" in _cache:
        return _cache["L1"], _cache["L2"]
    import jax
    cpu = jax.devices("cpu")[0]
    with jax.default_device(cpu):
        import jax.numpy as jnp
        k0, k1, k2 = jax.random.split(jax.random.key(42), 3)
        u0 = np.asarray(jax.random.uniform(k0, (B, D), dtype=jnp.float32))
        u1 = np.asarray(jax.random.uniform(k1, (E, B, H), dtype=jnp.float32))
        u2 = np.asarray(jax.random.uniform(k2, (E, B, H), dtype=jnp.float32))

    def lgt(u):
        with np.errstate(divide="ignore"):
            return ((np.log(u) - np.log1p(-u)) / SLOPE).astype(np.float32)

    # [E, B, H] -> [E][P, HB, B]   (value at [p, hb, b] = x[b, hb*P+p])
    def t(x):
        return np.ascontiguousarray(
            x.reshape(B, HB, P).transpose(2, 1, 0))

    L1 = [t(lgt(u1[e])) for e in range(E)]
    L2 = [t(lgt(u2[e])) for e in range(E)]
    _cache["L1"], _cache["L2"], _cache["u0"] = L1, L2, u0
    return L1, L2


def _tile_weights(w):
    """[H, D] fp32 -> hi/lo fp16 in [P, DC, HB, P] layout.

    lhsT tile for (dc, hb) is [:, dc, hb, :] = W.T[dc*P : dc*P+128, hb*P : ...]
    """
    wt = w.reshape(HB, P, DC, P).transpose(3, 2, 0, 1)  # [p(d), dc, hb, h]
    hi = wt.astype(np.float16)
    lo = (wt - hi.astype(np.float32)).astype(np.float16)
    return np.ascontiguousarray(hi), np.ascontiguousarray(lo)


def _build_program():
    if "nc" in _cache:
        return _cache["nc"]
    import concourse.mybir as mybir
    import concourse.tile as tile
    from concourse import bacc

    f16 = mybir.dt.float16
    f32 = mybir.dt.float32

    nc = bacc.Bacc(trn_type="TRN2", target_bir_lowering=False, num_devices=E)

    d_w1hi = nc.dram_tensor("w1hi", [P, DC, HB, P], f16, kind="ExternalInput").ap()
    d_w1lo = nc.dram_tensor("w1lo", [P, DC, HB, P], f16, kind="ExternalInput").ap()
    d_w2hi = nc.dram_tensor("w2hi", [P, HB, HB, P], f16, kind="ExternalInput").ap()
    d_w2lo = nc.dram_tensor("w2lo", [P, HB, HB, P], f16, kind="ExternalInput").ap()
    d_qs = nc.dram_tensor("qs", [P, DC, B], f16, kind="ExternalInput").ap()
    d_a1 = nc.dram_tensor("a1", [P, HB, B], f32, kind="ExternalInput").ap()
    d_a2 = nc.dram_tensor("a2", [P, HB, B], f32, kind="ExternalInput").ap()
    d_s2 = nc.dram_tensor("s2", [P, HB, B], f16, kind="ExternalOutput").ap()

    with tile.TileContext(nc) as tc:
        with tc.tile_pool(name="wpool", bufs=1) as wpool, \
             tc.tile_pool(name="xpool", bufs=2) as xpool, \
             tc.tile_pool(name="apool", bufs=4) as apool, \
             tc.tile_pool(name="spool", bufs=4) as spool, \
             tc.tile_pool(name="psum", bufs=4, space="PSUM") as psp, \
             tc.tile_pool(name="dram", bufs=1, space="DRAM") as dpool:

            s1d = dpool.tile([P, HB, B], f16)  # layer-1 spikes scratch

            def bass_ds(bt):
                return slice(bt * NB, (bt + 1) * NB)

            def gemm_spike_pass(whi_d, wlo_d, x_of_bt, a_d, out_d):
                w_hi = wpool.tile([P, DC, HB, P], f16, tag="whi")
                w_lo = wpool.tile([P, DC, HB, P], f16, tag="wlo")
                nc.gpsimd.dma_start(w_hi[:], whi_d)
                nc.gpsimd.dma_start(w_lo[:], wlo_d)
                for bt in range(BT):
                    bs = bass_ds(bt)
                    x = xpool.tile([P, DC, NB], f16, tag="x")
                    nc.gpsimd.dma_start(x[:], x_of_bt(bs))
                    for hb in range(HB):
                        ps = psp.tile([P, NB], f32, tag="ps")
                        for dc in range(DC):
                            nc.tensor.matmul(
                                ps[:], w_hi[:, dc, hb, :], x[:, dc, :],
                                start=(dc == 0), stop=False)
                        for dc in range(DC):
                            nc.tensor.matmul(
                                ps[:], w_lo[:, dc, hb, :], x[:, dc, :],
                                start=False, stop=(dc == DC - 1))
                        a = apool.tile([P, NB], f32, tag="a")
                        nc.gpsimd.dma_start(a[:], a_d[:, hb, bs])
                        s = spool.tile([P, NB], f16, tag="s")
                        nc.vector.tensor_tensor(
                            s[:], ps[:], a[:], op=mybir.AluOpType.is_gt)
                        nc.gpsimd.dma_start(out_d[:, hb, bs], s[:])

            gemm_spike_pass(d_w1hi, d_w1lo,
                            lambda bs: d_qs[:, :, bs], d_a1, s1d)
            gemm_spike_pass(d_w2hi, d_w2lo,
                            lambda bs: s1d[:, :, bs], d_a2, d_s2)

    nc.compile()
    _cache["nc"] = nc
    return nc


def _prepare(memory_query, W1, W2, th1, th2, Wg, bg):
    L1, L2 = _logit_thresholds()
    u0 = _cache["u0"]

    qs = (u0 < np.clip(memory_query, 0.0, 1.0)).astype(np.float32)  # [B, D]

    # gate on host (tiny)
    gl = qs @ Wg.T.astype(np.float32) + bg
    gl -= gl.max(axis=-1, keepdims=True)
    eg = np.exp(gl)
    gate = (eg / eg.sum(axis=-1, keepdims=True)).astype(np.float32)  # [B, E]

    qs_t = np.ascontiguousarray(
        qs.reshape(B, DC, P).transpose(2, 1, 0)).astype(np.float16)  # [P,DC,B]

    in_maps = []
    for e in range(E):
        w1hi, w1lo = _tile_weights(W1[e])
        w2hi, w2lo = _tile_weights(W2[e])
        a1 = L1[e] + th1[e].reshape(HB, P).T[:, :, None]   # [P, HB, B]
        a2 = L2[e] + th2[e].reshape(HB, P).T[:, :, None]
        in_maps.append({
            "w1hi": w1hi, "w1lo": w1lo, "w2hi": w2hi, "w2lo": w2lo,
            "qs": qs_t,
            "a1": np.ascontiguousarray(a1, dtype=np.float32),
            "a2": np.ascontiguousarray(a2, dtype=np.float32),
        })
    return in_maps, gate


def _install_ntff_shim():
    """This image's antenv lacks axon_hooks; graft it so trace=True works."""
    import sys
    import types
    try:
        from antenv.axon_hooks import get_axon_ntff_profile_hook  # noqa: F401
        return
    except ImportError:
        pass
    try:
        from trn_agent_boot.trn_boot import _ntff_profile_via_ctypes
        hook = _ntff_profile_via_ctypes("/opt/axon/libaxon_pjrt.so")
    except Exception:
        hook = None
    mod = types.ModuleType("antenv.axon_hooks")
    mod.get_axon_ntff_profile_hook = lambda: hook
    mod.set_axon_ntff_profile_hook = lambda h: None
    sys.modules["antenv.axon_hooks"] = mod


def kernel(memory_query, W1, W2, th1, th2, Wg, bg, _trace=False):
    from concourse import bass_utils

    if _trace:
        _install_ntff_shim()

    in_maps, gate = _prepare(memory_query, W1, W2, th1, th2, Wg, bg)
    nc = _build_program()
    res = bass_utils.run_bass_kernel_spmd(
        nc, in_maps, core_ids=list(range(E)), trace=_trace)
    _cache["last_results"] = res

    out = np.zeros((B, H), np.float32)
    for e in range(E):
        s2 = res.results[e]["s2"]  # [P, HB, B] f16
        s2_bh = s2.transpose(2, 1, 0).reshape(B, H).astype(np.float32)
        out += gate[:, e:e + 1] * s2_bh
    return out


# revision 7
# speedup vs baseline: 1.3036x; 1.3036x over previous
"""Trainium2 Bass kernel for the KnowledgeRetrievalNetwork (spiking MoE).

Strategy: expert-parallel across 8 NeuronCores (E=8, one expert per core).

The reference computes, with fixed PRNG key 42:
    qs   = bernoulli(k0, clip(mq,0,1))                    [B,D]
    gate = softmax(qs @ Wg.T + bg)                        [B,E]
    lin1 = qs @ W1[e].T ; s1 = bernoulli(k1, sigmoid(5*(lin1-th1)))
    lin2 = s1 @ W2[e].T ; s2 = bernoulli(k2, sigmoid(5*(lin2-th2)))
    out  = sum_e gate[:,e] * s2[e]                        [B,H]

jax's bernoulli(key,p) is `uniform(key,shape) < p` and threefry is
backend-invariant, so the uniform draws are input-independent constants.
We precompute them on host and fold them into per-element thresholds:
    u < sigmoid(SLOPE*(lin-th))  <=>  lin > th + logit(u)/SLOPE
The device kernel is then two dense GEMM+compare layers per expert,
computed in a transposed [feature, batch] layout so layer 2 consumes
layer 1's output directly (contraction on partitions, no transposes).

Weight precision: w = hi + lo with hi = fp16(w). The hi matmuls run in
fp16 at full PE rate. The lo residual is quantized to fp8-e4m3 scaled by
2^14 and run with perf_mode=DoubleRow (2 contraction chunks per matmul,
half the instructions). To let hi and lo accumulate into the SAME psum
bank, the hi weights are pre-scaled by 2^14 (exact in fp16) and the
comparison thresholds are pre-scaled by 2^14 on the host. Activations
(0/1 spikes) are exact in both fp16 and fp8.
"""

import numpy as np

B, D, H, E = 4096, 2048, 2048, 8
SLOPE = 5.0
P = 128                    # partitions
NB = 512                   # batch tile (one PSUM bank of fp32)
DC = D // P                # 16 contraction chunks (layer 1)
HB = H // P                # 16 output blocks
BT = B // NB               # 8 batch tiles

USE_FP8_LO = True
FP8_SCALE = 16384.0        # 2^14

_cache = {}


def _logit_thresholds():
    """Input-independent transposed logit(u)/SLOPE terms, [E][P, HB, B] f32."""
    if "L1" in _cache:
        return _cache["L1"], _cache["L2"]
    import jax
    cpu = jax.devices("cpu")[0]
    with jax.default_device(cpu):
        import jax.numpy as jnp
        k0, k1, k2 = jax.random.split(jax.random.key(42), 3)
        u0 = np.asarray(jax.random.uniform(k0, (B, D), dtype=jnp.float32))
        u1 = np.asarray(jax.random.uniform(k1, (E, B, H), dtype=jnp.float32))
        u2 = np.asarray(jax.random.uniform(k2, (E, B, H), dtype=jnp.float32))

    def lgt(u):
        with np.errstate(divide="ignore"):
            return ((np.log(u) - np.log1p(-u)) / SLOPE).astype(np.float32)

    # [E, B, H] -> [E][P, HB, B]   (value at [p, hb, b] = x[b, hb*P+p])
    def t(x):
        return np.ascontiguousarray(
            x.reshape(B, HB, P).transpose(2, 1, 0))

    L1 = [t(lgt(u1[e])) for e in range(E)]
    L2 = [t(lgt(u2[e])) for e in range(E)]
    _cache["L1"], _cache["L2"], _cache["u0"] = L1, L2, u0
    return L1, L2


def _tile_weights(w):
    """[H, D] fp32 -> (hi fp16, lo) in [P, DC, HB, P] lhsT-tile layout.

    lhsT tile for (dc, hb) is [:, dc, hb, :] = W.T[dc*P : dc*P+128, hb*P:...].
    With USE_FP8_LO, hi is pre-scaled by FP8_SCALE (exact in fp16) and lo is
    the residual scaled by FP8_SCALE in fp8-e4m3; otherwise lo is fp16.
    """
    import ml_dtypes
    wt = w.reshape(HB, P, DC, P).transpose(3, 2, 0, 1)  # [p(d), dc, hb, h]
    hi = wt.astype(np.float16)
    res = wt - hi.astype(np.float32)
    if USE_FP8_LO:
        hi_s = (hi.astype(np.float32) * FP8_SCALE).astype(np.float16)
        lo = (res * FP8_SCALE).astype(ml_dtypes.float8_e4m3)
        return np.ascontiguousarray(hi_s), np.ascontiguousarray(lo)
    lo = res.astype(np.float16)
    return np.ascontiguousarray(hi), np.ascontiguousarray(lo)


def _build_program():
    if "nc" in _cache:
        return _cache["nc"]
    import concourse.mybir as mybir
    import concourse.tile as tile
    from concourse import bacc

    f16 = mybir.dt.float16
    f32 = mybir.dt.float32
    f8 = mybir.dt.float8e4
    lo_dt = f8 if USE_FP8_LO else f16

    nc = bacc.Bacc(trn_type="TRN2", target_bir_lowering=False,
                   num_devices=E, num_swdge_queues=4)

    d_w1hi = nc.dram_tensor("w1hi", [P, DC, HB, P], f16, kind="ExternalInput").ap()
    d_w1lo = nc.dram_tensor("w1lo", [P, DC, HB, P], lo_dt, kind="ExternalInput").ap()
    d_w2hi = nc.dram_tensor("w2hi", [P, HB, HB, P], f16, kind="ExternalInput").ap()
    d_w2lo = nc.dram_tensor("w2lo", [P, HB, HB, P], lo_dt, kind="ExternalInput").ap()
    d_qs = nc.dram_tensor("qs", [P, DC, B], f16, kind="ExternalInput").ap()
    d_a1 = nc.dram_tensor("a1", [P, HB, B], f32, kind="ExternalInput").ap()
    d_a2 = nc.dram_tensor("a2", [P, HB, B], f32, kind="ExternalInput").ap()
    d_s2 = nc.dram_tensor("s2", [P, HB, B], f16, kind="ExternalOutput").ap()
    if USE_FP8_LO:
        d_qs8 = nc.dram_tensor("qs8", [P, DC, B], f8, kind="ExternalInput").ap()

    with tile.TileContext(nc) as tc:
        with tc.tile_pool(name="wpool", bufs=1) as wpool, \
             tc.tile_pool(name="xpool", bufs=2) as xpool, \
             tc.tile_pool(name="apool", bufs=4) as apool, \
             tc.tile_pool(name="spool", bufs=4) as spool, \
             tc.tile_pool(name="psum", bufs=4, space="PSUM") as psp, \
             tc.tile_pool(name="dram", bufs=1, space="DRAM") as dpool:

            s1d = dpool.tile([P, HB, B], f16)  # layer-1 spikes scratch
            s1d8 = None
            if USE_FP8_LO:
                s1d8 = dpool.tile([P, HB, B], f8, tag="s1d8")

            def bass_ds(bt):
                return slice(bt * NB, (bt + 1) * NB)

            def gemm_spike_pass(whi_d, wlo_d, x16_of, x8_of, a_d,
                                out_d, out8_d):
                # per-hb weight tiles: lets the next pass's weights stream
                # in while this pass finishes its last batch tile.
                whi, wlo = [], []
                for hb in range(HB):
                    th = wpool.tile([P, DC, P], f16, tag=f"whi{hb}")
                    tl = wpool.tile([P, DC, P], lo_dt, tag=f"wlo{hb}")
                    nc.gpsimd.dma_start(th[:], whi_d[:, :, hb, :])
                    nc.gpsimd.dma_start(tl[:], wlo_d[:, :, hb, :])
                    whi.append(th)
                    wlo.append(tl)
                for bt in range(BT):
                    bs = bass_ds(bt)
                    x = xpool.tile([P, DC, NB], f16, tag="x")
                    nc.gpsimd.dma_start(x[:], x16_of(bs))
                    if USE_FP8_LO:
                        x8 = xpool.tile([P, DC, NB], f8, tag="x8")
                        nc.gpsimd.dma_start(x8[:], x8_of(bs))
                    for hb in range(HB):
                        ps = psp.tile([P, NB], f32, tag="ps")
                        for dc in range(DC):
                            nc.tensor.matmul(
                                ps[:], whi[hb][:, dc, :], x[:, dc, :],
                                start=(dc == 0), stop=False)
                        if USE_FP8_LO:
                            for i in range(DC // 2):
                                nc.tensor.matmul(
                                    ps[:],
                                    wlo[hb][:, 2 * i:2 * i + 2, :],
                                    x8[:, 2 * i:2 * i + 2, :],
                                    start=False, stop=(i == DC // 2 - 1),
                                    perf_mode=mybir.MatmulPerfMode.DoubleRow)
                        else:
                            for dc in range(DC):
                                nc.tensor.matmul(
                                    ps[:], wlo[hb][:, dc, :], x[:, dc, :],
                                    start=False, stop=(dc == DC - 1))
                        a = apool.tile([P, NB], f32, tag="a")
                        nc.gpsimd.dma_start(a[:], a_d[:, hb, bs])
                        s = spool.tile([P, NB], f16, tag="s")
                        nc.vector.tensor_tensor(
                            s[:], ps[:], a[:], op=mybir.AluOpType.is_gt)
                        nc.gpsimd.dma_start(out_d[:, hb, bs], s[:])
                        if out8_d is not None:
                            s8 = spool.tile([P, NB], f8, tag="s8")
                            nc.vector.tensor_tensor(
                                s8[:], ps[:], a[:], op=mybir.AluOpType.is_gt)
                            nc.gpsimd.dma_start(out8_d[:, hb, bs], s8[:])

            gemm_spike_pass(
                d_w1hi, d_w1lo,
                lambda bs: d_qs[:, :, bs],
                (lambda bs: d_qs8[:, :, bs]) if USE_FP8_LO else None,
                d_a1, s1d, s1d8)
            gemm_spike_pass(
                d_w2hi, d_w2lo,
                lambda bs: s1d[:, :, bs],
                (lambda bs: s1d8[:, :, bs]) if USE_FP8_LO else None,
                d_a2, d_s2, None)

    nc.compile()
    _cache["nc"] = nc
    return nc


def _prepare(memory_query, W1, W2, th1, th2, Wg, bg):
    import ml_dtypes
    L1, L2 = _logit_thresholds()
    u0 = _cache["u0"]

    qs = (u0 < np.clip(memory_query, 0.0, 1.0)).astype(np.float32)  # [B, D]

    # gate on host (tiny)
    gl = qs @ Wg.T.astype(np.float32) + bg
    gl -= gl.max(axis=-1, keepdims=True)
    eg = np.exp(gl)
    gate = (eg / eg.sum(axis=-1, keepdims=True)).astype(np.float32)  # [B, E]

    qs_t = np.ascontiguousarray(
        qs.reshape(B, DC, P).transpose(2, 1, 0))                 # [P, DC, B]
    qs_16 = qs_t.astype(np.float16)

    a_scale = FP8_SCALE if USE_FP8_LO else 1.0

    in_maps = []
    for e in range(E):
        w1hi, w1lo = _tile_weights(W1[e])
        w2hi, w2lo = _tile_weights(W2[e])
        a1 = (L1[e] + th1[e].reshape(HB, P).T[:, :, None]) * a_scale
        a2 = (L2[e] + th2[e].reshape(HB, P).T[:, :, None]) * a_scale
        im = {
            "w1hi": w1hi, "w1lo": w1lo, "w2hi": w2hi, "w2lo": w2lo,
            "qs": qs_16,
            "a1": np.ascontiguousarray(a1, dtype=np.float32),
            "a2": np.ascontiguousarray(a2, dtype=np.float32),
        }
        if USE_FP8_LO:
            im["qs8"] = qs_t.astype(ml_dtypes.float8_e4m3)
        in_maps.append(im)
    return in_maps, gate


def _install_ntff_shim():
    """This image's antenv lacks axon_hooks; graft it so trace=True works."""
    import sys
    import types
    try:
        from antenv.axon_hooks import get_axon_ntff_profile_hook  # noqa: F401
        return
    except ImportError:
        pass
    try:
        from trn_agent_boot.trn_boot import _ntff_profile_via_ctypes
        hook = _ntff_profile_via_ctypes("/opt/axon/libaxon_pjrt.so")
    except Exception:
        hook = None
    mod = types.ModuleType("antenv.axon_hooks")
    mod.get_axon_ntff_profile_hook = lambda: hook
    mod.set_axon_ntff_profile_hook = lambda h: None
    sys.modules["antenv.axon_hooks"] = mod


def kernel(memory_query, W1, W2, th1, th2, Wg, bg, _trace=False):
    from concourse import bass_utils

    if _trace:
        _install_ntff_shim()

    in_maps, gate = _prepare(memory_query, W1, W2, th1, th2, Wg, bg)
    nc = _build_program()
    res = bass_utils.run_bass_kernel_spmd(
        nc, in_maps, core_ids=list(range(E)), trace=_trace)
    _cache["last_results"] = res

    out = np.zeros((B, H), np.float32)
    for e in range(E):
        s2 = res.results[e]["s2"]  # [P, HB, B] f16
        s2_bh = s2.transpose(2, 1, 0).reshape(B, H).astype(np.float32)
        out += gate[:, e:e + 1] * s2_bh
    return out


# revision 10
# speedup vs baseline: 1.3246x; 1.0161x over previous
"""Trainium2 Bass kernel for the KnowledgeRetrievalNetwork (spiking MoE).

Strategy: expert-parallel across 8 NeuronCores (E=8, one expert per core).

The reference computes, with fixed PRNG key 42:
    qs   = bernoulli(k0, clip(mq,0,1))                    [B,D]
    gate = softmax(qs @ Wg.T + bg)                        [B,E]
    lin1 = qs @ W1[e].T ; s1 = bernoulli(k1, sigmoid(5*(lin1-th1)))
    lin2 = s1 @ W2[e].T ; s2 = bernoulli(k2, sigmoid(5*(lin2-th2)))
    out  = sum_e gate[:,e] * s2[e]                        [B,H]

jax's bernoulli(key,p) is `uniform(key,shape) < p` and threefry is
backend-invariant, so the uniform draws are input-independent constants.
We precompute them on host and fold them into per-element thresholds:
    u < sigmoid(SLOPE*(lin-th))  <=>  lin > th + logit(u)/SLOPE
The device kernel is then two dense GEMM+compare layers per expert,
computed in a transposed [feature, batch] layout so layer 2 consumes
layer 1's output directly (contraction on partitions, no transposes).

Weight precision: w = hi + lo with hi = fp16(w). The hi matmuls run in
fp16 at full PE rate. The lo residual is quantized to fp8-e4m3 scaled by
2^14 and run with perf_mode=DoubleRow (2 contraction chunks per matmul,
half the instructions). To let hi and lo accumulate into the SAME psum
bank, the hi weights are pre-scaled by 2^14 (exact in fp16) and the
comparison thresholds are pre-scaled by 2^14 on the host. Activations
(0/1 spikes) are exact in both fp16 and fp8.
"""

import numpy as np

B, D, H, E = 4096, 2048, 2048, 8
SLOPE = 5.0
P = 128                    # partitions
NB = 512                   # batch tile (one PSUM bank of fp32)
DC = D // P                # 16 contraction chunks (layer 1)
HB = H // P                # 16 output blocks
BT = B // NB               # 8 batch tiles

USE_FP8_LO = True
FP8_SCALE = 16384.0        # 2^14

_cache = {}


def _logit_thresholds():
    """Input-independent transposed logit(u)/SLOPE terms, [E][P, HB, B] f32."""
    if "L1" in _cache:
        return _cache["L1"], _cache["L2"]
    import jax
    cpu = jax.devices("cpu")[0]
    with jax.default_device(cpu):
        import jax.numpy as jnp
        k0, k1, k2 = jax.random.split(jax.random.key(42), 3)
        u0 = np.asarray(jax.random.uniform(k0, (B, D), dtype=jnp.float32))
        u1 = np.asarray(jax.random.uniform(k1, (E, B, H), dtype=jnp.float32))
        u2 = np.asarray(jax.random.uniform(k2, (E, B, H), dtype=jnp.float32))

    def lgt(u):
        with np.errstate(divide="ignore"):
            return ((np.log(u) - np.log1p(-u)) / SLOPE).astype(np.float32)

    # [E, B, H] -> [E][P, HB, B]   (value at [p, hb, b] = x[b, hb*P+p])
    def t(x):
        return np.ascontiguousarray(
            x.reshape(B, HB, P).transpose(2, 1, 0))

    L1 = [t(lgt(u1[e])) for e in range(E)]
    L2 = [t(lgt(u2[e])) for e in range(E)]
    _cache["L1"], _cache["L2"], _cache["u0"] = L1, L2, u0
    return L1, L2


def _tile_weights(w):
    """[H, D] fp32 -> (hi fp16, lo) in [P, DC, HB, P] lhsT-tile layout.

    lhsT tile for (dc, hb) is [:, dc, hb, :] = W.T[dc*P : dc*P+128, hb*P:...].
    With USE_FP8_LO, hi is pre-scaled by FP8_SCALE (exact in fp16) and lo is
    the residual scaled by FP8_SCALE in fp8-e4m3; otherwise lo is fp16.
    """
    import ml_dtypes
    wt = w.reshape(HB, P, DC, P).transpose(3, 2, 0, 1)  # [p(d), dc, hb, h]
    hi = wt.astype(np.float16)
    res = wt - hi.astype(np.float32)
    if USE_FP8_LO:
        hi_s = (hi.astype(np.float32) * FP8_SCALE).astype(np.float16)
        lo = (res * FP8_SCALE).astype(ml_dtypes.float8_e4m3)
        return np.ascontiguousarray(hi_s), np.ascontiguousarray(lo)
    lo = res.astype(np.float16)
    return np.ascontiguousarray(hi), np.ascontiguousarray(lo)


def _build_program():
    if "nc" in _cache:
        return _cache["nc"]
    import concourse.mybir as mybir
    import concourse.tile as tile
    from concourse import bacc

    f16 = mybir.dt.float16
    f32 = mybir.dt.float32
    f8 = mybir.dt.float8e4
    lo_dt = f8 if USE_FP8_LO else f16

    nc = bacc.Bacc(trn_type="TRN2", target_bir_lowering=False,
                   num_devices=E, num_swdge_queues=4)

    d_w1hi = nc.dram_tensor("w1hi", [P, DC, HB, P], f16, kind="ExternalInput").ap()
    d_w1lo = nc.dram_tensor("w1lo", [P, DC, HB, P], lo_dt, kind="ExternalInput").ap()
    d_w2hi = nc.dram_tensor("w2hi", [P, HB, HB, P], f16, kind="ExternalInput").ap()
    d_w2lo = nc.dram_tensor("w2lo", [P, HB, HB, P], lo_dt, kind="ExternalInput").ap()
    d_qs = nc.dram_tensor("qs", [P, DC, B], f16, kind="ExternalInput").ap()
    d_a1 = nc.dram_tensor("a1", [P, HB, B], f32, kind="ExternalInput").ap()
    d_a2 = nc.dram_tensor("a2", [P, HB, B], f32, kind="ExternalInput").ap()
    d_s2 = nc.dram_tensor("s2", [P, HB, B], f16, kind="ExternalOutput").ap()
    if USE_FP8_LO:
        d_qs8 = nc.dram_tensor("qs8", [P, DC, B], f8, kind="ExternalInput").ap()

    with tile.TileContext(nc) as tc:
        with tc.tile_pool(name="wpool", bufs=1) as wpool, \
             tc.tile_pool(name="xpool", bufs=2) as xpool, \
             tc.tile_pool(name="apool", bufs=4) as apool, \
             tc.tile_pool(name="spool", bufs=4) as spool, \
             tc.tile_pool(name="psum", bufs=4, space="PSUM") as psp, \
             tc.tile_pool(name="dram", bufs=1, space="DRAM") as dpool:

            # layer-1 spikes scratch, split per batch-tile so pass 2's
            # reads only wait on the matching pass-1 writes.
            s1d = [dpool.tile([P, HB, NB], f16, name=f"s1d{bt}", tag=f"s1d{bt}")
                   for bt in range(BT)]
            s1d8 = None
            if USE_FP8_LO:
                s1d8 = [dpool.tile([P, HB, NB], f8, name=f"s1d8{bt}", tag=f"s1d8{bt}")
                        for bt in range(BT)]

            def gemm_spike_pass(whi_d, wlo_d, x16_of, x8_of, a_d, out_of):
                # first batch tile's activations prefetch ahead of the
                # bulk weight load so the PE can start immediately.
                x = xpool.tile([P, DC, NB], f16, tag="x")
                nc.gpsimd.dma_start(x[:], x16_of(0))
                if USE_FP8_LO:
                    x8 = xpool.tile([P, DC, NB], f8, tag="x8")
                    nc.gpsimd.dma_start(x8[:], x8_of(0))
                # per-hb weight tiles: lets the next pass's weights stream
                # in while this pass finishes its last batch tile.
                whi, wlo = [], []
                for hb in range(HB):
                    th = wpool.tile([P, DC, P], f16, tag=f"whi{hb}")
                    tl = wpool.tile([P, DC, P], lo_dt, tag=f"wlo{hb}")
                    nc.gpsimd.dma_start(th[:], whi_d[:, :, hb, :])
                    nc.gpsimd.dma_start(tl[:], wlo_d[:, :, hb, :])
                    whi.append(th)
                    wlo.append(tl)
                for bt in range(BT):
                    bs = slice(bt * NB, (bt + 1) * NB)
                    if bt > 0:
                        x = xpool.tile([P, DC, NB], f16, tag="x")
                        nc.gpsimd.dma_start(x[:], x16_of(bt))
                        if USE_FP8_LO:
                            x8 = xpool.tile([P, DC, NB], f8, tag="x8")
                            nc.gpsimd.dma_start(x8[:], x8_of(bt))
                    for hb in range(HB):
                        ps = psp.tile([P, NB], f32, tag="ps")
                        for dc in range(DC):
                            nc.tensor.matmul(
                                ps[:], whi[hb][:, dc, :], x[:, dc, :],
                                start=(dc == 0), stop=False)
                        if USE_FP8_LO:
                            for i in range(DC // 2):
                                nc.tensor.matmul(
                                    ps[:],
                                    wlo[hb][:, 2 * i:2 * i + 2, :],
                                    x8[:, 2 * i:2 * i + 2, :],
                                    start=False, stop=(i == DC // 2 - 1),
                                    perf_mode=mybir.MatmulPerfMode.DoubleRow)
                        else:
                            for dc in range(DC):
                                nc.tensor.matmul(
                                    ps[:], wlo[hb][:, dc, :], x[:, dc, :],
                                    start=False, stop=(dc == DC - 1))
                        a = apool.tile([P, NB], f32, tag="a")
                        nc.gpsimd.dma_start(a[:], a_d[:, hb, bs])
                        for dst, sdt, stag in out_of(hb, bt):
                            s = spool.tile([P, NB], sdt, tag=stag)
                            nc.vector.tensor_tensor(
                                s[:], ps[:], a[:], op=mybir.AluOpType.is_gt)
                            nc.gpsimd.dma_start(dst, s[:])

            def pass1_out(hb, bt):
                outs = [(s1d[bt][:, hb, :], f16, "s")]
                if USE_FP8_LO:
                    outs.append((s1d8[bt][:, hb, :], f8, "s8"))
                return outs

            def pass2_out(hb, bt):
                bs = slice(bt * NB, (bt + 1) * NB)
                return [(d_s2[:, hb, bs], f16, "s")]

            gemm_spike_pass(
                d_w1hi, d_w1lo,
                lambda bt: d_qs[:, :, slice(bt * NB, (bt + 1) * NB)],
                (lambda bt: d_qs8[:, :, slice(bt * NB, (bt + 1) * NB)])
                if USE_FP8_LO else None,
                d_a1, pass1_out)
            gemm_spike_pass(
                d_w2hi, d_w2lo,
                lambda bt: s1d[bt][:],
                (lambda bt: s1d8[bt][:]) if USE_FP8_LO else None,
                d_a2, pass2_out)

    nc.compile()
    _cache["nc"] = nc
    return nc


def _prepare(memory_query, W1, W2, th1, th2, Wg, bg):
    import ml_dtypes
    L1, L2 = _logit_thresholds()
    u0 = _cache["u0"]

    qs = (u0 < np.clip(memory_query, 0.0, 1.0)).astype(np.float32)  # [B, D]

    # gate on host (tiny)
    gl = qs @ Wg.T.astype(np.float32) + bg
    gl -= gl.max(axis=-1, keepdims=True)
    eg = np.exp(gl)
    gate = (eg / eg.sum(axis=-1, keepdims=True)).astype(np.float32)  # [B, E]

    qs_t = np.ascontiguousarray(
        qs.reshape(B, DC, P).transpose(2, 1, 0))                 # [P, DC, B]
    qs_16 = qs_t.astype(np.float16)

    a_scale = FP8_SCALE if USE_FP8_LO else 1.0

    in_maps = []
    for e in range(E):
        w1hi, w1lo = _tile_weights(W1[e])
        w2hi, w2lo = _tile_weights(W2[e])
        a1 = (L1[e] + th1[e].reshape(HB, P).T[:, :, None]) * a_scale
        a2 = (L2[e] + th2[e].reshape(HB, P).T[:, :, None]) * a_scale
        im = {
            "w1hi": w1hi, "w1lo": w1lo, "w2hi": w2hi, "w2lo": w2lo,
            "qs": qs_16,
            "a1": np.ascontiguousarray(a1, dtype=np.float32),
            "a2": np.ascontiguousarray(a2, dtype=np.float32),
        }
        if USE_FP8_LO:
            im["qs8"] = qs_t.astype(ml_dtypes.float8_e4m3)
        in_maps.append(im)
    return in_maps, gate


def _install_ntff_shim():
    """This image's antenv lacks axon_hooks; graft it so trace=True works."""
    import sys
    import types
    try:
        from antenv.axon_hooks import get_axon_ntff_profile_hook  # noqa: F401
        return
    except ImportError:
        pass
    try:
        from trn_agent_boot.trn_boot import _ntff_profile_via_ctypes
        hook = _ntff_profile_via_ctypes("/opt/axon/libaxon_pjrt.so")
    except Exception:
        hook = None
    mod = types.ModuleType("antenv.axon_hooks")
    mod.get_axon_ntff_profile_hook = lambda: hook
    mod.set_axon_ntff_profile_hook = lambda h: None
    sys.modules["antenv.axon_hooks"] = mod


def kernel(memory_query, W1, W2, th1, th2, Wg, bg, _trace=False):
    from concourse import bass_utils

    if _trace:
        _install_ntff_shim()

    in_maps, gate = _prepare(memory_query, W1, W2, th1, th2, Wg, bg)
    nc = _build_program()
    res = bass_utils.run_bass_kernel_spmd(
        nc, in_maps, core_ids=list(range(E)), trace=_trace)
    _cache["last_results"] = res

    out = np.zeros((B, H), np.float32)
    for e in range(E):
        s2 = res.results[e]["s2"]  # [P, HB, B] f16
        s2_bh = s2.transpose(2, 1, 0).reshape(B, H).astype(np.float32)
        out += gate[:, e:e + 1] * s2_bh
    return out


# revision 11
# speedup vs baseline: 1.3446x; 1.0151x over previous
"""Trainium2 Bass kernel for the KnowledgeRetrievalNetwork (spiking MoE).

Strategy: expert-parallel across 8 NeuronCores (E=8, one expert per core).

The reference computes, with fixed PRNG key 42:
    qs   = bernoulli(k0, clip(mq,0,1))                    [B,D]
    gate = softmax(qs @ Wg.T + bg)                        [B,E]
    lin1 = qs @ W1[e].T ; s1 = bernoulli(k1, sigmoid(5*(lin1-th1)))
    lin2 = s1 @ W2[e].T ; s2 = bernoulli(k2, sigmoid(5*(lin2-th2)))
    out  = sum_e gate[:,e] * s2[e]                        [B,H]

jax's bernoulli(key,p) is `uniform(key,shape) < p` and threefry is
backend-invariant, so the uniform draws are input-independent constants.
We precompute them on host and fold them into per-element thresholds:
    u < sigmoid(SLOPE*(lin-th))  <=>  lin > th + logit(u)/SLOPE
The device kernel is then two dense GEMM+compare layers per expert,
computed in a transposed [feature, batch] layout so layer 2 consumes
layer 1's output directly (contraction on partitions, no transposes).

Weight precision: w = hi + lo with hi = fp16(w). The hi matmuls run in
fp16 at full PE rate. The lo residual is quantized to fp8-e4m3 scaled by
2^14 and run with perf_mode=DoubleRow (2 contraction chunks per matmul,
half the instructions). To let hi and lo accumulate into the SAME psum
bank, the hi weights are pre-scaled by 2^14 (exact in fp16) and the
comparison thresholds are pre-scaled by 2^14 on the host. Activations
(0/1 spikes) are exact in both fp16 and fp8.
"""

import numpy as np

B, D, H, E = 4096, 2048, 2048, 8
SLOPE = 5.0
P = 128                    # partitions
NB = 512                   # batch tile (one PSUM bank of fp32)
DC = D // P                # 16 contraction chunks (layer 1)
HB = H // P                # 16 output blocks
BT = B // NB               # 8 batch tiles

USE_FP8_LO = True
FP8_SCALE = 16384.0        # 2^14

_cache = {}


def _logit_thresholds():
    """Input-independent transposed logit(u)/SLOPE terms, [E][P, HB, B] f32."""
    if "L1" in _cache:
        return _cache["L1"], _cache["L2"]
    import jax
    cpu = jax.devices("cpu")[0]
    with jax.default_device(cpu):
        import jax.numpy as jnp
        k0, k1, k2 = jax.random.split(jax.random.key(42), 3)
        u0 = np.asarray(jax.random.uniform(k0, (B, D), dtype=jnp.float32))
        u1 = np.asarray(jax.random.uniform(k1, (E, B, H), dtype=jnp.float32))
        u2 = np.asarray(jax.random.uniform(k2, (E, B, H), dtype=jnp.float32))

    def lgt(u):
        with np.errstate(divide="ignore"):
            return ((np.log(u) - np.log1p(-u)) / SLOPE).astype(np.float32)

    # [E, B, H] -> [E][P, HB, B]   (value at [p, hb, b] = x[b, hb*P+p])
    def t(x):
        return np.ascontiguousarray(
            x.reshape(B, HB, P).transpose(2, 1, 0))

    L1 = [t(lgt(u1[e])) for e in range(E)]
    L2 = [t(lgt(u2[e])) for e in range(E)]
    _cache["L1"], _cache["L2"], _cache["u0"] = L1, L2, u0
    return L1, L2


def _tile_weights(w):
    """[H, D] fp32 -> (hi fp16, lo) in [P, DC, HB, P] lhsT-tile layout.

    lhsT tile for (dc, hb) is [:, dc, hb, :] = W.T[dc*P : dc*P+128, hb*P:...].
    With USE_FP8_LO, hi is pre-scaled by FP8_SCALE (exact in fp16) and lo is
    the residual scaled by FP8_SCALE in fp8-e4m3; otherwise lo is fp16.
    """
    import ml_dtypes
    wt = w.reshape(HB, P, DC, P).transpose(3, 2, 0, 1)  # [p(d), dc, hb, h]
    hi = wt.astype(np.float16)
    res = wt - hi.astype(np.float32)
    if USE_FP8_LO:
        hi_s = (hi.astype(np.float32) * FP8_SCALE).astype(np.float16)
        lo = (res * FP8_SCALE).astype(ml_dtypes.float8_e4m3)
        return np.ascontiguousarray(hi_s), np.ascontiguousarray(lo)
    lo = res.astype(np.float16)
    return np.ascontiguousarray(hi), np.ascontiguousarray(lo)


def _build_program():
    if "nc" in _cache:
        return _cache["nc"]
    import concourse.mybir as mybir
    import concourse.tile as tile
    from concourse import bacc

    f16 = mybir.dt.float16
    f32 = mybir.dt.float32
    f8 = mybir.dt.float8e4
    lo_dt = f8 if USE_FP8_LO else f16

    nc = bacc.Bacc(trn_type="TRN2", target_bir_lowering=False,
                   num_devices=E, num_swdge_queues=4)

    d_w1hi = nc.dram_tensor("w1hi", [P, DC, HB, P], f16, kind="ExternalInput").ap()
    d_w1lo = nc.dram_tensor("w1lo", [P, DC, HB, P], lo_dt, kind="ExternalInput").ap()
    d_w2hi = nc.dram_tensor("w2hi", [P, HB, HB, P], f16, kind="ExternalInput").ap()
    d_w2lo = nc.dram_tensor("w2lo", [P, HB, HB, P], lo_dt, kind="ExternalInput").ap()
    d_qs = nc.dram_tensor("qs", [P, DC, B], f16, kind="ExternalInput").ap()
    d_a1 = nc.dram_tensor("a1", [P, HB, B], f32, kind="ExternalInput").ap()
    d_a2 = nc.dram_tensor("a2", [P, HB, B], f32, kind="ExternalInput").ap()
    d_s2 = nc.dram_tensor("s2", [P, HB, B], f16, kind="ExternalOutput").ap()
    if USE_FP8_LO:
        d_qs8 = nc.dram_tensor("qs8", [P, DC, B], f8, kind="ExternalInput").ap()

    with tile.TileContext(nc) as tc:
        with tc.tile_pool(name="wpool", bufs=1) as wpool, \
             tc.tile_pool(name="xpool", bufs=2) as xpool, \
             tc.tile_pool(name="apool", bufs=6) as apool, \
             tc.tile_pool(name="spool", bufs=6) as spool, \
             tc.tile_pool(name="psum", bufs=6, space="PSUM") as psp, \
             tc.tile_pool(name="dram", bufs=1, space="DRAM") as dpool:

            # layer-1 spikes scratch, split per batch-tile so pass 2's
            # reads only wait on the matching pass-1 writes.
            s1d = [dpool.tile([P, HB, NB], f16, name=f"s1d{bt}", tag=f"s1d{bt}")
                   for bt in range(BT)]
            s1d8 = None
            if USE_FP8_LO:
                s1d8 = [dpool.tile([P, HB, NB], f8, name=f"s1d8{bt}", tag=f"s1d8{bt}")
                        for bt in range(BT)]

            def gemm_spike_pass(whi_d, wlo_d, x16_of, x8_of, a_d, out_of):
                # first batch tile's activations prefetch ahead of the
                # bulk weight load so the PE can start immediately.
                x = xpool.tile([P, DC, NB], f16, tag="x")
                nc.gpsimd.dma_start(x[:], x16_of(0))
                if USE_FP8_LO:
                    x8 = xpool.tile([P, DC, NB], f8, tag="x8")
                    nc.gpsimd.dma_start(x8[:], x8_of(0))
                # per-hb weight tiles: lets the next pass's weights stream
                # in while this pass finishes its last batch tile.
                whi, wlo = [], []
                for hb in range(HB):
                    th = wpool.tile([P, DC, P], f16, tag=f"whi{hb}")
                    tl = wpool.tile([P, DC, P], lo_dt, tag=f"wlo{hb}")
                    nc.gpsimd.dma_start(th[:], whi_d[:, :, hb, :])
                    nc.gpsimd.dma_start(tl[:], wlo_d[:, :, hb, :])
                    whi.append(th)
                    wlo.append(tl)
                for bt in range(BT):
                    bs = slice(bt * NB, (bt + 1) * NB)
                    if bt > 0:
                        x = xpool.tile([P, DC, NB], f16, tag="x")
                        nc.gpsimd.dma_start(x[:], x16_of(bt))
                        if USE_FP8_LO:
                            x8 = xpool.tile([P, DC, NB], f8, tag="x8")
                            nc.gpsimd.dma_start(x8[:], x8_of(bt))
                    for hb in range(HB):
                        ps = psp.tile([P, NB], f32, tag="ps")
                        for dc in range(DC):
                            nc.tensor.matmul(
                                ps[:], whi[hb][:, dc, :], x[:, dc, :],
                                start=(dc == 0), stop=False)
                        if USE_FP8_LO:
                            for i in range(DC // 2):
                                nc.tensor.matmul(
                                    ps[:],
                                    wlo[hb][:, 2 * i:2 * i + 2, :],
                                    x8[:, 2 * i:2 * i + 2, :],
                                    start=False, stop=(i == DC // 2 - 1),
                                    perf_mode=mybir.MatmulPerfMode.DoubleRow)
                        else:
                            for dc in range(DC):
                                nc.tensor.matmul(
                                    ps[:], wlo[hb][:, dc, :], x[:, dc, :],
                                    start=False, stop=(dc == DC - 1))
                        a = apool.tile([P, NB], f32, tag="a")
                        nc.gpsimd.dma_start(a[:], a_d[:, hb, bs])
                        for dst, sdt, stag in out_of(hb, bt):
                            s = spool.tile([P, NB], sdt, tag=stag)
                            nc.vector.tensor_tensor(
                                s[:], ps[:], a[:], op=mybir.AluOpType.is_gt)
                            nc.gpsimd.dma_start(dst, s[:])

            def pass1_out(hb, bt):
                outs = [(s1d[bt][:, hb, :], f16, "s")]
                if USE_FP8_LO:
                    outs.append((s1d8[bt][:, hb, :], f8, "s8"))
                return outs

            def pass2_out(hb, bt):
                bs = slice(bt * NB, (bt + 1) * NB)
                return [(d_s2[:, hb, bs], f16, "s")]

            gemm_spike_pass(
                d_w1hi, d_w1lo,
                lambda bt: d_qs[:, :, slice(bt * NB, (bt + 1) * NB)],
                (lambda bt: d_qs8[:, :, slice(bt * NB, (bt + 1) * NB)])
                if USE_FP8_LO else None,
                d_a1, pass1_out)
            gemm_spike_pass(
                d_w2hi, d_w2lo,
                lambda bt: s1d[bt][:],
                (lambda bt: s1d8[bt][:]) if USE_FP8_LO else None,
                d_a2, pass2_out)

    nc.compile()
    _cache["nc"] = nc
    return nc


def _prepare(memory_query, W1, W2, th1, th2, Wg, bg):
    import ml_dtypes
    L1, L2 = _logit_thresholds()
    u0 = _cache["u0"]

    qs = (u0 < np.clip(memory_query, 0.0, 1.0)).astype(np.float32)  # [B, D]

    # gate on host (tiny)
    gl = qs @ Wg.T.astype(np.float32) + bg
    gl -= gl.max(axis=-1, keepdims=True)
    eg = np.exp(gl)
    gate = (eg / eg.sum(axis=-1, keepdims=True)).astype(np.float32)  # [B, E]

    qs_t = np.ascontiguousarray(
        qs.reshape(B, DC, P).transpose(2, 1, 0))                 # [P, DC, B]
    qs_16 = qs_t.astype(np.float16)

    a_scale = FP8_SCALE if USE_FP8_LO else 1.0

    in_maps = []
    for e in range(E):
        w1hi, w1lo = _tile_weights(W1[e])
        w2hi, w2lo = _tile_weights(W2[e])
        a1 = (L1[e] + th1[e].reshape(HB, P).T[:, :, None]) * a_scale
        a2 = (L2[e] + th2[e].reshape(HB, P).T[:, :, None]) * a_scale
        im = {
            "w1hi": w1hi, "w1lo": w1lo, "w2hi": w2hi, "w2lo": w2lo,
            "qs": qs_16,
            "a1": np.ascontiguousarray(a1, dtype=np.float32),
            "a2": np.ascontiguousarray(a2, dtype=np.float32),
        }
        if USE_FP8_LO:
            im["qs8"] = qs_t.astype(ml_dtypes.float8_e4m3)
        in_maps.append(im)
    return in_maps, gate


def _install_ntff_shim():
    """This image's antenv lacks axon_hooks; graft it so trace=True works."""
    import sys
    import types
    try:
        from antenv.axon_hooks import get_axon_ntff_profile_hook  # noqa: F401
        return
    except ImportError:
        pass
    try:
        from trn_agent_boot.trn_boot import _ntff_profile_via_ctypes
        hook = _ntff_profile_via_ctypes("/opt/axon/libaxon_pjrt.so")
    except Exception:
        hook = None
    mod = types.ModuleType("antenv.axon_hooks")
    mod.get_axon_ntff_profile_hook = lambda: hook
    mod.set_axon_ntff_profile_hook = lambda h: None
    sys.modules["antenv.axon_hooks"] = mod


def kernel(memory_query, W1, W2, th1, th2, Wg, bg, _trace=False):
    from concourse import bass_utils

    if _trace:
        _install_ntff_shim()

    in_maps, gate = _prepare(memory_query, W1, W2, th1, th2, Wg, bg)
    nc = _build_program()
    res = bass_utils.run_bass_kernel_spmd(
        nc, in_maps, core_ids=list(range(E)), trace=_trace)
    _cache["last_results"] = res

    out = np.zeros((B, H), np.float32)
    for e in range(E):
        s2 = res.results[e]["s2"]  # [P, HB, B] f16
        s2_bh = s2.transpose(2, 1, 0).reshape(B, H).astype(np.float32)
        out += gate[:, e:e + 1] * s2_bh
    return out
